# revision 34
# baseline (speedup 1.0000x reference)
"""Multi-head causal attention with RoPE on 8 Trainium2 NeuronCores.

Sharding: data-parallel over batch (2) x tensor-parallel over heads (16 -> 4
per core). Each core computes q/k/v projections for its 4 heads on its batch
element, attention, and a partial output projection (its rows of wo); the
host sums the 4 partials per batch element.

Device-side layout trick: everything is computed "transposed" (scores held as
[t, s]) so no on-device transposes are needed anywhere:
  - host supplies x^T (bf16), so projections produce q^T/k^T [head_dim, s]
    directly and v [t, head_dim] directly
  - softmax denominators come from an all-ones matmul (partition reduction on
    the tensor engine); 1/x is a fast reciprocal on the vector engine
  - attn^T [n, s] is exactly the lhsT the wo matmul wants
RoPE pair-swap is done by permuting the wq/wk columns on the host into
(even|odd) half-layout so the swap becomes two partition-halved SBUF->SBUF
DMA copies instead of cross-lane compute.

Causal-variant optimizations over the original version:
  - diagonal 128x512 score tiles are computed singly and column-trimmed to
    their causal extent ([128*p:512]); the triangular 128x128 edge block is
    masked by multiplying the exp output with a constant {0,1} tri tile on
    the vector engine, eliminating all identity-matmul mask accumulations
    from the tensor engine
  - chunk-0 q/k projections run d-outer across all 8 PSUM banks so the first
    matmuls start as soon as the first 128KB weight piece lands; DMA pieces
    are sized/ordered to match consumption
  - next-chunk projections are emitted interleaved into the attention head
    loop (with the previous chunk's wo units) so the tensor engine never
    drains behind softmax chains
  - cos/sin inputs, rope temporaries and output partials are bf16 (halves
    the out DMA; host sums partials in f32)

kernel() inspects the mask and dispatches to the tuned causal build, or to a
legacy build for full attention with no mask / an arbitrary additive mask.
"""

import math

import ml_dtypes
import numpy as np

import concourse.bass as bass
import concourse.mybir as mybir
import concourse.tile as tile
from concourse import bacc
from concourse.bass_utils import run_bass_kernel_spmd

BF16 = ml_dtypes.bfloat16
F32 = mybir.dt.float32
BF = mybir.dt.bfloat16
AF = mybir.ActivationFunctionType

N_CORES = 8
B = 2
S = 2048
D = 2048
H = 16
HD = 128
H_LOC = 4          # heads per core
N_LOC = H_LOC * HD  # 512 local head dims
NJ = 4             # s-chunks
SC = S // NJ       # 512 s-chunk width
DCH = D // 128     # 16 contraction chunks
SCALE = 1.0 / math.sqrt(HD)

_BUILDS: dict = {}
LAST_RESULT = None


def _build_causal():
    nc = bacc.Bacc("TRN2", target_bir_lowering=False, debug=False,
                   num_devices=N_CORES)

    xt_d = nc.dram_tensor("xt", [NJ, 128, DCH, SC], BF, kind="ExternalInput").ap()
    wq_d = nc.dram_tensor("wq", [128, DCH, N_LOC], BF, kind="ExternalInput").ap()
    wk_d = nc.dram_tensor("wk", [128, DCH, N_LOC], BF, kind="ExternalInput").ap()
    wv_d = nc.dram_tensor("wv", [128, DCH, N_LOC], BF, kind="ExternalInput").ap()
    wo_d = nc.dram_tensor("wo", [128, H_LOC, D], BF, kind="ExternalInput").ap()
    cose_d = nc.dram_tensor("cose", [128, S], BF, kind="ExternalInput").ap()
    sine_d = nc.dram_tensor("sine", [128, S], BF, kind="ExternalInput").ap()
    tri01_d = nc.dram_tensor("tri01", [128, 128], BF, kind="ExternalInput").ap()
    out_d = nc.dram_tensor("out", [S, D], BF, kind="ExternalOutput").ap()
    out_v = out_d.rearrange("(a p) d -> a p d", p=128)

    with tile.TileContext(nc) as tc:
        with (
            tc.tile_pool(name="singles", bufs=1) as singles,
            tc.tile_pool(name="doubles", bufs=2) as doubles,
            tc.tile_pool(name="triples", bufs=3) as triples,
            tc.tile_pool(name="quads", bufs=4) as quads,
            tc.tile_pool(name="ps1", bufs=1, space="PSUM") as ps1,
            tc.tile_pool(name="ps2", bufs=2, space="PSUM") as ps2,
        ):
            # ---- persistent tensors ----
            wq_sb = singles.tile([128, DCH, N_LOC], BF, tag="wq")
            wk_sb = singles.tile([128, DCH, N_LOC], BF, tag="wk")
            wv_sb = singles.tile([128, DCH, N_LOC], BF, tag="wv")
            wo_sb = singles.tile([128, H_LOC, D], BF, tag="wo")
            cose_sb = singles.tile([128, S], BF, tag="cose")
            sine_sb = singles.tile([128, S], BF, tag="sine")
            tri01_sb = singles.tile([128, 128], BF, tag="tri01")
            ktrot = singles.tile([128, H_LOC, S], BF, tag="ktrot")
            v_sb = singles.tile([128, NJ * H_LOC, SC], BF, tag="v")
            xt0_sb = doubles.tile([128, DCH, SC], BF, tag="xt")

            # DMA emission in consumption order: chunk-0 q/k run d-outer with
            # wq on the sync queue and wk on the (otherwise idle) scalar
            # queue so both stream in parallel; first pieces small so the PE
            # starts early; xt chunk 0 on gpsimd with matching pieces
            PIECES = ((0, 1), (1, 3), (3, 6), (6, 10), (10, 13), (13, 16))
            for a, b in PIECES:
                nc.sync.dma_start(out=wq_sb[:, a:b, :], in_=wq_d[:, a:b, :])
                nc.scalar.dma_start(out=wk_sb[:, a:b, :], in_=wk_d[:, a:b, :])
                nc.gpsimd.dma_start(out=xt0_sb[:, a:b, :], in_=xt_d[0][:, a:b, :])
            nc.sync.dma_start(out=cose_sb[:, 0:SC], in_=cose_d[:, 0:SC])
            nc.sync.dma_start(out=sine_sb[:, 0:SC], in_=sine_d[:, 0:SC])
            nc.sync.dma_start(out=tri01_sb[:], in_=tri01_d[:])
            nc.sync.dma_start(out=wv_sb[:, 0:8, :], in_=wv_d[:, 0:8, :])
            nc.sync.dma_start(out=wv_sb[:, 8:16, :], in_=wv_d[:, 8:16, :])
            nc.sync.dma_start(out=cose_sb[:, SC:], in_=cose_d[:, SC:])
            nc.sync.dma_start(out=sine_sb[:, SC:], in_=sine_d[:, SC:])
            nc.sync.dma_start(out=wo_sb[:], in_=wo_d[:])
            ones_sb = singles.tile([128, 128], BF, tag="ones")
            nc.vector.memset(ones_sb[:], 1.0)

            # ---- PE warmup: the HAM clock gate starts at 1.2GHz and only
            # reaches 2.4GHz after ~3.4us of sustained matmul activity. The
            # PE sits idle during the initial DMA ramp anyway, so burn that
            # time on dummy matmuls and take the cold penalty for free.
            warm_sb = singles.tile([128, SC], BF, tag="warm")
            nc.vector.memset(warm_sb[:], 0.0)
            warm_ps = ps2.tile([128, 2, SC], F32, tag="sc", name="warm_ps")
            for _ in range(12):
                nc.tensor.matmul(warm_ps[:, 0, :], ones_sb[:], warm_sb[:],
                                 start=True, stop=True)

            def rope_group(items):
                """items: list of (psum AP, dest AP, js). Emits all muls (and
                swap DMAs) before any add so the DVE never head-of-line
                blocks a PSUM-freeing mul behind a DMA-waiting add."""
                pend = []
                for ps_ap, dst, js in items:
                    a_sb = quads.tile([128, SC], BF, tag="ropeA")
                    nc.vector.tensor_mul(a_sb[:], ps_ap, cose_sb[:, js:js + SC])
                    b_sb = quads.tile([128, SC], BF, tag="ropeB")
                    nc.vector.tensor_mul(b_sb[:], ps_ap, sine_sb[:, js:js + SC])
                    b2_sb = quads.tile([128, SC], BF, tag="ropeB2")
                    nc.gpsimd.dma_start(out=b2_sb[0:64, :], in_=b_sb[64:128, :])
                    nc.gpsimd.dma_start(out=b2_sb[64:128, :], in_=b_sb[0:64, :])
                    pend.append((dst, a_sb, b2_sb))
                for dst, a_sb, b2_sb in pend:
                    nc.vector.tensor_add(dst, a_sb[:], b2_sb[:])

            def proj_qk_pair(j, xt_sb, w_sb, hp, qdest):
                """q or k projections + rope for one head pair of chunk j."""
                js = j * SC
                items = []
                for hh in range(2):
                    h = hp * 2 + hh
                    ps = ps2.tile([128, SC], F32, tag="qkv1")
                    for d in range(DCH):
                        nc.tensor.matmul(
                            ps[:],
                            w_sb[:, d, h * 128:(h + 1) * 128],
                            xt_sb[:, d, :],
                            start=(d == 0), stop=(d == DCH - 1),
                        )
                    if qdest is not None:
                        dst = qdest[:, h, :]
                    else:
                        dst = ktrot[:, h, js:js + SC]
                    items.append((ps[:], dst, js))
                rope_group(items)

            def proj_v(j, xt_sb):
                for tl in range(4):
                    ps = ps2.tile([128, SC], F32, tag="qkv1")
                    for d in range(DCH):
                        nc.tensor.matmul(
                            ps[:],
                            xt_sb[:, d, tl * 128:(tl + 1) * 128],
                            wv_sb[:, d, :],
                            start=(d == 0), stop=(d == DCH - 1),
                        )
                    nc.scalar.copy(out=v_sb[:, 4 * j + tl, :], in_=ps[:])

            def wo_units(j, attnT_j, st, dcs, deep=False):
                # stage the s-block's 4 units in one tile: a single 512KB DMA
                # with 4KB-contiguous runs per partition instead of 4 strided
                # 128KB DMAs
                o4 = triples.tile([128, 4, SC], BF, tag="ostage")
                for dc in dcs:
                    if deep and dc % 2 == 1:
                        # end-of-kernel block: no attention running, so the
                        # sc banks are free — use them for a deeper pipeline
                        wide = ps2.tile([128, 2, SC], F32, tag="sc",
                                        name="wo_wide")
                        wps = wide[:, 0, :]
                    else:
                        wnarrow = ps2.tile([128, SC], F32, tag="qkv1",
                                           name="wo_wps")
                        wps = wnarrow[:]
                    for h2 in range(H_LOC):
                        nc.tensor.matmul(
                            wps,
                            attnT_j[:, h2, st * 128:(st + 1) * 128],
                            wo_sb[:, h2, dc * SC:(dc + 1) * SC],
                            start=(h2 == 0), stop=(h2 == H_LOC - 1),
                        )
                    if (st + dc) % 2 == 0:
                        nc.scalar.copy(out=o4[:, dc, :], in_=wps)
                    else:
                        nc.vector.tensor_copy(o4[:, dc, :], wps)
                    if deep:
                        # final block: per-unit DMAs so the drain after the
                        # last matmul is one 128KB transfer, not 512KB
                        nc.sync.dma_start(
                            out=out_v[4 * j + st][:, dc * SC:(dc + 1) * SC],
                            in_=o4[:, dc, :])
                if not deep:
                    nc.sync.dma_start(out=out_v[4 * j + st], in_=o4[:])

            def att_head(j, h, qt, attnT_j):
                """Attention for head h of chunk j. qt: [128, SC] rotated q.

                Diagonal tiles first (longest chains), column-trimmed to the
                causal extent; triangular edge masked by a {0,1} multiply on
                DVE after exp. Full tiles follow at full width.
                """
                sums_ps = ps1.tile([128, SC], F32, tag="sums")
                pv_ps = ps1.tile([128, SC], F32, tag="pv")
                n_fp = 2 * j  # full (off-diagonal) t-tile pairs
                dexp = []
                for half in range(2):
                    sc_ps = ps2.tile([128, 2, SC], F32, tag="sc")
                    exp_sb = triples.tile([128, 2, SC], BF, tag="exp")
                    for i_ in range(2):
                        p = 2 * half + i_
                        tt = 4 * j + p
                        w0 = p * 128
                        nc.tensor.matmul(
                            sc_ps[:, i_, w0:SC],
                            ktrot[:, h, tt * 128:(tt + 1) * 128],
                            qt[:, w0:SC],
                            start=True, stop=True,
                        )
                        if p < 3:
                            nc.scalar.activation(
                                out=exp_sb[:, i_, w0 + 128:SC],
                                in_=sc_ps[:, i_, w0 + 128:SC],
                                func=AF.Exp, scale=SCALE)
                        bt = quads.tile([128, 128], BF, tag="bt")
                        nc.scalar.activation(
                            out=bt[:], in_=sc_ps[:, i_, w0:w0 + 128],
                            func=AF.Exp, scale=SCALE)
                        nc.vector.tensor_mul(
                            exp_sb[:, i_, w0:w0 + 128], bt[:], tri01_sb[:])
                    for i_ in range(2):
                        p = 2 * half + i_
                        tt = 4 * j + p
                        w0 = p * 128
                        nc.tensor.matmul(
                            pv_ps[:, w0:SC],
                            v_sb[:, tt, h * 128:(h + 1) * 128],
                            exp_sb[:, i_, w0:SC],
                            start=(p == 0), stop=(j == 0 and p == 3),
                            skip_group_check=True,
                        )
                    dexp.append(exp_sb)
                # piecewise exp-sum of the trimmed diag tiles (valid columns
                # of tile p are [128p:512]; untouched regions hold garbage)
                t01 = triples.tile([128, SC], BF, tag="t01")
                nc.vector.tensor_copy(t01[:, 0:128], dexp[0][:, 0, 0:128])
                nc.vector.tensor_add(t01[:, 128:SC], dexp[0][:, 0, 128:SC],
                                     dexp[0][:, 1, 128:SC])
                t23 = triples.tile([128, SC], BF, tag="t23")
                nc.vector.tensor_copy(t23[:, 256:384], dexp[1][:, 0, 256:384])
                nc.vector.tensor_add(t23[:, 384:SC], dexp[1][:, 0, 384:SC],
                                     dexp[1][:, 1, 384:SC])
                # single full-width ones-matmul: a second start=True subrange
                # piece on the same bank would reset has_written bank-wide
                # and break later accumulation
                efin = triples.tile([128, SC], BF, tag="efin")
                nc.vector.tensor_copy(efin[:, 0:256], t01[:, 0:256])
                nc.vector.tensor_add(efin[:, 256:SC], t01[:, 256:SC],
                                     t23[:, 256:SC])
                nc.tensor.matmul(sums_ps[:], ones_sb[:], efin[:],
                                 start=True, stop=(j == 0),
                                 skip_group_check=True)
                # ---- full tiles ----
                prev_epair = None
                for fp in range(n_fp):
                    sc_ps = ps2.tile([128, 2, SC], F32, tag="sc")
                    exp_sb = triples.tile([128, 2, SC], BF, tag="exp")
                    for i_ in range(2):
                        tt = fp * 2 + i_
                        nc.tensor.matmul(
                            sc_ps[:, i_, :],
                            ktrot[:, h, tt * 128:(tt + 1) * 128],
                            qt[:],
                            start=True, stop=True,
                        )
                    nc.scalar.activation(out=exp_sb[:], in_=sc_ps[:],
                                         func=AF.Exp, scale=SCALE)
                    epair = triples.tile([128, SC], BF, tag="epair")
                    nc.vector.tensor_add(epair[:], exp_sb[:, 0, :],
                                         exp_sb[:, 1, :])
                    if fp % 2 == 0:
                        prev_epair = epair
                    else:
                        equad = triples.tile([128, SC], BF, tag="equad")
                        nc.vector.tensor_add(equad[:], prev_epair[:], epair[:])
                        nc.tensor.matmul(sums_ps[:], ones_sb[:], equad[:],
                                         start=False, stop=(fp == n_fp - 1),
                                         skip_group_check=True)
                    for i_ in range(2):
                        tt = fp * 2 + i_
                        nc.tensor.matmul(
                            pv_ps[:],
                            v_sb[:, tt, h * 128:(h + 1) * 128],
                            exp_sb[:, i_, :],
                            start=False, stop=(fp == n_fp - 1 and i_ == 1),
                            skip_group_check=True,
                        )
                recip_sb = doubles.tile([128, SC], F32, tag="recip")
                nc.vector.reciprocal_approx_fast(out=recip_sb[:], in_=sums_ps[:])
                nc.vector.tensor_mul(attnT_j[:, h, :], pv_ps[:], recip_sb[:])

            # ---- chunk 0 projections in two waves so the PE never waits on
            # the serial rope chains: wave A (q x4 + k h0/h1) runs d-outer
            # over 6 PSUM banks tracking the streaming DMA pieces; wave B
            # (k h2/h3) lands in the still-free second sc buffer and keeps
            # the PE busy while wave A's ropes drain on DVE ----
            qps = [ps2.tile([128, SC], F32, tag="qkv1", name="qps0"),
                   ps2.tile([128, SC], F32, tag="qkv1", name="qps1"),
                   ps1.tile([128, SC], F32, tag="sums", name="qps2"),
                   ps1.tile([128, SC], F32, tag="pv", name="qps3")]
            kpsA = ps2.tile([128, 2, SC], F32, tag="sc", name="kpsA")
            for d in range(DCH):
                for h in range(H_LOC):
                    nc.tensor.matmul(qps[h][:],
                                     wq_sb[:, d, h * 128:(h + 1) * 128],
                                     xt0_sb[:, d, :],
                                     start=(d == 0), stop=(d == DCH - 1))
                for h in range(2):
                    nc.tensor.matmul(kpsA[:, h, :],
                                     wk_sb[:, d, h * 128:(h + 1) * 128],
                                     xt0_sb[:, d, :],
                                     start=(d == 0), stop=(d == DCH - 1))
                if 1 <= d <= 8:
                    # filler during DMA piece waits: keeps the HAM activity
                    # window busy so the PE clock never re-throttles while
                    # wave A is input-bandwidth paced
                    nc.tensor.matmul(warm_ps[:, 0, :], ones_sb[:], warm_sb[:],
                                     start=True, stop=True)
            qtrot0 = doubles.tile([128, H_LOC, SC], BF, tag="qtrot")
            rope_group([(qps[h][:], qtrot0[:, h, :], 0) for h in range(H_LOC)])
            rope_group([(kpsA[:, h, :], ktrot[:, h, 0:SC], 0)
                        for h in range(2)])
            kpsB = ps2.tile([128, 2, SC], F32, tag="sc", name="kpsB")
            for d in range(DCH):
                for h in range(2):
                    nc.tensor.matmul(kpsB[:, h, :],
                                     wk_sb[:, d, (h + 2) * 128:(h + 3) * 128],
                                     xt0_sb[:, d, :],
                                     start=(d == 0), stop=(d == DCH - 1))
            rope_group([(kpsB[:, h, :], ktrot[:, h + 2, 0:SC], 0)
                        for h in range(2)])
            proj_v(0, xt0_sb)

            # ---- attention chunks with interleaved next-chunk projections
            # and previous-chunk wo units filling softmax-chain bubbles ----
            prev_attnT = None
            xt_cur, qt_cur = xt0_sb, qtrot0
            for j in range(NJ):
                attnT_j = doubles.tile([128, H_LOC, SC], BF, tag="attnT")
                if j + 1 < NJ:
                    xt_next = doubles.tile([128, DCH, SC], BF, tag="xt")
                    for a, b in ((0, 4), (4, 8), (8, 12), (12, 16)):
                        nc.gpsimd.dma_start(out=xt_next[:, a:b, :],
                                            in_=xt_d[j + 1][:, a:b, :])
                    qt_next = doubles.tile([128, H_LOC, SC], BF, tag="qtrot")
                for h in range(H_LOC):
                    att_head(j, h, qt_cur[:, h, :], attnT_j)
                    if prev_attnT is not None:
                        wo_units(j - 1, prev_attnT, h, range(4))
                    if j + 1 < NJ:
                        if h < 2:
                            proj_qk_pair(j + 1, xt_next, wq_sb, h, qt_next)
                        else:
                            proj_qk_pair(j + 1, xt_next, wk_sb, h - 2, None)
                if j + 1 < NJ:
                    proj_v(j + 1, xt_next)
                    xt_cur, qt_cur = xt_next, qt_next
                prev_attnT = attnT_j
            for st in range(4):
                wo_units(NJ - 1, prev_attnT, st, range(4), deep=True)

    nc.compile()
    return nc


def _build_legacy(variant: str, nj: int = NJ):
    """variant in {'full_nomask', 'full_mask'} (original implementation)."""
    use_mask = variant == "full_mask"

    nc = bacc.Bacc("TRN2", target_bir_lowering=False, debug=False,
                   num_devices=N_CORES)

    xt_d = nc.dram_tensor("xt", [NJ, 128, DCH, SC], BF, kind="ExternalInput").ap()
    wq_d = nc.dram_tensor("wq", [128, DCH, N_LOC], BF, kind="ExternalInput").ap()
    wk_d = nc.dram_tensor("wk", [128, DCH, N_LOC], BF, kind="ExternalInput").ap()
    wv_d = nc.dram_tensor("wv", [128, DCH, N_LOC], BF, kind="ExternalInput").ap()
    wo_d = nc.dram_tensor("wo", [128, H_LOC, D], BF, kind="ExternalInput").ap()
    cose_d = nc.dram_tensor("cose", [128, S], F32, kind="ExternalInput").ap()
    sine_d = nc.dram_tensor("sine", [128, S], F32, kind="ExternalInput").ap()
    maskt_d = None
    if use_mask:
        maskt_d = nc.dram_tensor("maskt", [NJ, 128, DCH, SC], BF,
                                 kind="ExternalInput").ap()
    out_d = nc.dram_tensor("out", [S, D], F32, kind="ExternalOutput").ap()
    out_v = out_d.rearrange("(a p) d -> a p d", p=128)

    with tile.TileContext(nc) as tc:
        with (
            tc.tile_pool(name="singles", bufs=1) as singles,
            tc.tile_pool(name="doubles", bufs=2) as doubles,
            tc.tile_pool(name="triples", bufs=3) as triples,
            tc.tile_pool(name="ps1", bufs=1, space="PSUM") as ps1,
            tc.tile_pool(name="ps2", bufs=2, space="PSUM") as ps2,
        ):
            rope_pool = doubles if use_mask else triples
            stage_pool = doubles if use_mask else triples
            epair_pool = doubles
            # ---- constants / persistent tensors ----
            wq_sb = singles.tile([128, DCH, N_LOC], BF, tag="wq")
            wk_sb = singles.tile([128, DCH, N_LOC], BF, tag="wk")
            wv_sb = singles.tile([128, DCH, N_LOC], BF, tag="wv")
            wo_sb = singles.tile([128, H_LOC, D], BF, tag="wo")
            xt_pool = singles if use_mask else doubles
            xt0_sb = xt_pool.tile([128, DCH, SC], BF, tag="xt")
            for q4 in range(4):
                nc.sync.dma_start(out=wq_sb[:, 4 * q4:4 * (q4 + 1), :],
                                  in_=wq_d[:, 4 * q4:4 * (q4 + 1), :])
                nc.gpsimd.dma_start(out=xt0_sb[:, 4 * q4:4 * (q4 + 1), :],
                                    in_=xt_d[0][:, 4 * q4:4 * (q4 + 1), :])
            cose_sb = singles.tile([128, S], F32, tag="cose")
            sine_sb = singles.tile([128, S], F32, tag="sine")
            nc.sync.dma_start(out=cose_sb[:], in_=cose_d[:])
            nc.sync.dma_start(out=sine_sb[:], in_=sine_d[:])
            nc.sync.dma_start(out=wk_sb[:], in_=wk_d[:])
            nc.sync.dma_start(out=wv_sb[:], in_=wv_d[:])
            nc.sync.dma_start(out=wo_sb[:], in_=wo_d[:])
            from concourse.masks import make_identity
            ones_sb = singles.tile([128, 128], BF, tag="ones")
            nc.vector.memset(ones_sb[:], 1.0)
            ident_sb = singles.tile([128, 128], BF, tag="ident")
            make_identity(nc, ident_sb[:])
            ktrot = singles.tile([128, H_LOC, S], BF, tag="ktrot")
            v_sb = singles.tile([128, NJ * H_LOC, SC], BF, tag="v")
            qtrot_all = singles.tile([128, H_LOC, S], BF, tag="qtrot_all")

            def projections(j, qdest, qsl, xt_pre=None):
                js = j * SC
                if xt_pre is not None:
                    xt_sb = xt_pre
                else:
                    xt_sb = xt_pool.tile([128, DCH, SC], BF, tag="xt")
                    nc.gpsimd.dma_start(out=xt_sb[:], in_=xt_d[j])

                for w_sb, dest, dsl in ((wq_sb, qdest, qsl),
                                        (wk_sb, ktrot, slice(js, js + SC))):
                    for hp in range(2):
                        parts = []
                        for hh in range(2):
                            h = hp * 2 + hh
                            if j == 0 and hp == 1:
                                ps = ps1.tile([128, SC], F32,
                                              tag="pv" if hh else "sums")
                            else:
                                ps = ps2.tile([128, SC], F32, tag="qkv1")
                            for d in range(DCH):
                                nc.tensor.matmul(
                                    ps[:],
                                    w_sb[:, d, h * 128:(h + 1) * 128],
                                    xt_sb[:, d, :],
                                    start=(d == 0), stop=(d == DCH - 1),
                                )
                            a_sb = rope_pool.tile([128, SC], F32, tag="ropeA")
                            nc.vector.tensor_mul(
                                a_sb[:], ps[:], cose_sb[:, js:js + SC])
                            b_sb = triples.tile([128, SC], F32, tag="ropeB")
                            nc.vector.tensor_mul(
                                b_sb[:], ps[:], sine_sb[:, js:js + SC])
                            b2_sb = triples.tile([128, SC], F32, tag="ropeB2")
                            nc.scalar.dma_start(out=b2_sb[0:64, :],
                                                in_=b_sb[64:128, :])
                            nc.scalar.dma_start(out=b2_sb[64:128, :],
                                                in_=b_sb[0:64, :])
                            parts.append((h, a_sb, b2_sb))
                        for h, a_sb, b2_sb in parts:
                            if dsl is None:
                                dst = dest[:, h, :]
                            else:
                                dst = dest[:, h, dsl]
                            nc.vector.tensor_add(dst, a_sb[:], b2_sb[:])

                for tl in range(4):
                    ps = ps2.tile([128, SC], F32, tag="qkv1")
                    for d in range(DCH):
                        nc.tensor.matmul(
                            ps[:],
                            xt_sb[:, d, tl * 128:(tl + 1) * 128],
                            wv_sb[:, d, :],
                            start=(d == 0), stop=(d == DCH - 1),
                        )
                    nc.scalar.copy(out=v_sb[:, 4 * j + tl, :], in_=ps[:])

            def wo_units(j, attnT_j, st, dcs):
                for dc in dcs:
                    wps = ps2.tile([128, SC], F32, tag="qkv1")
                    for h2 in range(H_LOC):
                        nc.tensor.matmul(
                            wps[:],
                            attnT_j[:, h2, st * 128:(st + 1) * 128],
                            wo_sb[:, h2, dc * SC:(dc + 1) * SC],
                            start=(h2 == 0), stop=(h2 == H_LOC - 1),
                        )
                    o_sb = stage_pool.tile([128, SC], F32, tag="ostage")
                    if (st + dc) % 2 == 0:
                        nc.scalar.copy(out=o_sb[:], in_=wps[:])
                    else:
                        nc.vector.tensor_copy(o_sb[:], wps[:])
                    nc.sync.dma_start(
                        out=out_v[4 * j + st][:, dc * SC:(dc + 1) * SC],
                        in_=o_sb[:])

            def attention_and_wo(j, qtrot_h, prev=None):
                maskt_sb = None
                if use_mask:
                    maskt_sb = xt_pool.tile([128, DCH, SC], BF, tag="xt")
                    nc.sync.dma_start(out=maskt_sb[:], in_=maskt_d[j])

                attnT_j = doubles.tile([128, H_LOC, SC], BF, tag="attnT")
                pg_order = list(range(DCH // 2))
                for h in range(H_LOC):
                    sums_ps = ps1.tile([128, SC], F32, tag="sums")
                    pv_ps = ps1.tile([128, SC], F32, tag="pv")
                    for gi, pg in enumerate(pg_order):
                        sc_ps = ps2.tile([128, 2, SC], F32, tag="sc")
                        exp_sb = stage_pool.tile([128, 2, SC], BF, tag="exp")
                        for i_ in range(2):
                            tt = pg * 2 + i_
                            nc.tensor.matmul(
                                sc_ps[:, i_, :],
                                ktrot[:, h, tt * 128:(tt + 1) * 128],
                                qtrot_h(h),
                                start=True, stop=not use_mask,
                            )
                            if use_mask:
                                nc.tensor.matmul(
                                    sc_ps[:, i_, :], ident_sb[:],
                                    maskt_sb[:, tt, :],
                                    start=False, stop=True,
                                )
                        nc.scalar.activation(out=exp_sb[:], in_=sc_ps[:],
                                             func=AF.Exp, scale=SCALE)
                        epair = epair_pool.tile([128, SC], BF, tag="epair")
                        nc.vector.tensor_add(epair[:], exp_sb[:, 0, :],
                                             exp_sb[:, 1, :])
                        nc.tensor.matmul(sums_ps[:], ones_sb[:], epair[:],
                                         start=gi == 0,
                                         stop=gi == len(pg_order) - 1)
                        for i_ in range(2):
                            tt = pg * 2 + i_
                            first = gi == 0 and i_ == 0
                            last = gi == len(pg_order) - 1 and i_ == 1
                            nc.tensor.matmul(pv_ps[:],
                                             v_sb[:, tt, h * 128:(h + 1) * 128],
                                             exp_sb[:, i_, :],
                                             start=first, stop=last)
                    recip_sb = doubles.tile([128, SC], F32, tag="recip")
                    nc.vector.reciprocal_approx_fast(out=recip_sb[:], in_=sums_ps[:])
                    nc.vector.tensor_mul(attnT_j[:, h, :], pv_ps[:], recip_sb[:])
                    if prev is not None:
                        wo_units(j - 1, prev, h, range(4))
                return attnT_j

            pending = None
            for j in range(nj):
                projections(j, qtrot_all, slice(j * SC, (j + 1) * SC),
                            xt_pre=xt0_sb if j == 0 else None)
            for j in range(nj):
                js = j * SC
                pending = attention_and_wo(
                    j, lambda h, js=js: qtrot_all[:, h, js:js + SC],
                    prev=pending)
            if pending is not None:
                for st in range(4):
                    wo_units(nj - 1, pending, st, range(4))

    nc.compile()
    return nc


def _get_build(variant):
    if variant not in _BUILDS:
        if variant == "causal":
            _BUILDS[variant] = _build_causal()
        else:
            _BUILDS[variant] = _build_legacy(variant)
    return _BUILDS[variant]


def _classify_mask(mask):
    if not np.any(mask):
        return "full_nomask"
    tril = np.tril(np.ones((S, S), dtype=bool))
    if np.all(mask[tril] == 0.0) and np.all(mask[~tril] <= -1e9):
        return "causal"
    return "full_mask"


def kernel(x, wq, wk, wv, wo, freqs_cos, freqs_sin, mask):
    global LAST_RESULT
    x = np.asarray(x)
    wq, wk, wv, wo = (np.asarray(w) for w in (wq, wk, wv, wo))
    freqs_cos = np.asarray(freqs_cos, dtype=np.float32)
    freqs_sin = np.asarray(freqs_sin, dtype=np.float32)
    mask = np.asarray(mask, dtype=np.float32)

    variant = _classify_mask(mask)
    causal = variant == "causal"
    nc = _get_build(variant)

    # half-layout column permutation within each head (even indices then odd)
    perm = np.concatenate([np.arange(0, 128, 2), np.arange(1, 128, 2)])

    def wproj_arr(w, g):
        cols = w[:, 512 * g:512 * (g + 1)].reshape(D, H_LOC, 128)
        cols = cols[:, :, perm].reshape(D, N_LOC)
        return np.ascontiguousarray(
            cols.reshape(DCH, 128, N_LOC).transpose(1, 0, 2)).astype(BF16)

    def wv_arr(w, g):
        cols = w[:, 512 * g:512 * (g + 1)]
        return np.ascontiguousarray(
            cols.reshape(DCH, 128, N_LOC).transpose(1, 0, 2)).astype(BF16)

    def wo_arr(g):
        rows = wo[512 * g:512 * (g + 1), :]
        return np.ascontiguousarray(
            rows.reshape(H_LOC, 128, D).transpose(1, 0, 2)).astype(BF16)

    # cos/sin in half-layout: rows j and j+64 carry pair j's cos; sine rows
    # 0..63 = +sin (source a_j -> target j+64), rows 64..127 = -sin
    cs_dt = BF16 if causal else np.float32
    cosE = np.empty((128, S), cs_dt)
    sinE = np.empty((128, S), cs_dt)
    cosE[0:64] = freqs_cos.T
    cosE[64:128] = freqs_cos.T
    sinE[0:64] = freqs_sin.T
    sinE[64:128] = -freqs_sin.T

    xt_b = []
    for b in range(B):
        xT = x[b].T.astype(BF16)  # [D, S]
        xt = np.ascontiguousarray(
            xT.reshape(DCH, 128, NJ, SC).transpose(2, 1, 0, 3))
        xt_b.append(xt)

    tri01 = None
    if causal:
        r = np.arange(128)
        tri01 = (r[:, None] <= r[None, :]).astype(BF16)

    maskt = None
    if variant == "full_mask":
        # exp computes exp(SCALE * (scores + m')) with m' = mask^T / SCALE
        mT = (mask.T / SCALE).astype(BF16)  # [t, s]
        maskt = np.ascontiguousarray(
            mT.reshape(DCH, 128, NJ, SC).transpose(2, 1, 0, 3))

    wq_g = [wproj_arr(wq, g) for g in range(H_LOC)]
    wk_g = [wproj_arr(wk, g) for g in range(H_LOC)]
    wv_g = [wv_arr(wv, g) for g in range(H_LOC)]
    wo_g = [wo_arr(g) for g in range(H_LOC)]

    in_maps = []
    for c in range(N_CORES):
        b, g = c // 4, c % 4
        m = {
            "xt": xt_b[b],
            "wq": wq_g[g], "wk": wk_g[g], "wv": wv_g[g], "wo": wo_g[g],
            "cose": cosE, "sine": sinE,
        }
        if tri01 is not None:
            m["tri01"] = tri01
        if maskt is not None:
            m["maskt"] = maskt
        in_maps.append(m)

    res = run_bass_kernel_spmd(nc, in_maps, list(range(N_CORES)))
    LAST_RESULT = res

    outs = [np.asarray(res.results[c]["out"], dtype=np.float32)
            for c in range(N_CORES)]
    out = np.stack([
        outs[0] + outs[1] + outs[2] + outs[3],
        outs[4] + outs[5] + outs[6] + outs[7],
    ])
    return out


# revision 35
# speedup vs baseline: 1.0079x; 1.0079x over previous
"""Multi-head causal attention with RoPE on 8 Trainium2 NeuronCores.

Sharding: data-parallel over batch (2) x tensor-parallel over heads (16 -> 4
per core). Each core computes q/k/v projections for its 4 heads on its batch
element, attention, and a partial output projection (its rows of wo); the
host sums the 4 partials per batch element.

Device-side layout trick: everything is computed "transposed" (scores held as
[t, s]) so no on-device transposes are needed anywhere:
  - host supplies x^T (bf16), so projections produce q^T/k^T [head_dim, s]
    directly and v [t, head_dim] directly
  - softmax denominators come from an all-ones matmul (partition reduction on
    the tensor engine); 1/x is a fast reciprocal on the vector engine
  - attn^T [n, s] is exactly the lhsT the wo matmul wants
RoPE pair-swap is done by permuting the wq/wk columns on the host into
(even|odd) half-layout so the swap becomes two partition-halved SBUF->SBUF
DMA copies instead of cross-lane compute.

Causal-variant optimizations over the original version:
  - diagonal 128x512 score tiles are computed singly and column-trimmed to
    their causal extent ([128*p:512]); the triangular 128x128 edge block is
    masked by multiplying the exp output with a constant {0,1} tri tile on
    the vector engine, eliminating all identity-matmul mask accumulations
    from the tensor engine
  - chunk-0 q/k projections run d-outer across all 8 PSUM banks so the first
    matmuls start as soon as the first 128KB weight piece lands; DMA pieces
    are sized/ordered to match consumption
  - next-chunk projections are emitted interleaved into the attention head
    loop (with the previous chunk's wo units) so the tensor engine never
    drains behind softmax chains
  - cos/sin inputs, rope temporaries and output partials are bf16 (halves
    the out DMA; host sums partials in f32)

kernel() inspects the mask and dispatches to the tuned causal build, or to a
legacy build for full attention with no mask / an arbitrary additive mask.
"""

import math

import ml_dtypes
import numpy as np

import concourse.bass as bass
import concourse.mybir as mybir
import concourse.tile as tile
from concourse import bacc
from concourse.bass_utils import run_bass_kernel_spmd

BF16 = ml_dtypes.bfloat16
F32 = mybir.dt.float32
BF = mybir.dt.bfloat16
AF = mybir.ActivationFunctionType

N_CORES = 8
B = 2
S = 2048
D = 2048
H = 16
HD = 128
H_LOC = 4          # heads per core
N_LOC = H_LOC * HD  # 512 local head dims
NJ = 4             # s-chunks
SC = S // NJ       # 512 s-chunk width
DCH = D // 128     # 16 contraction chunks
SCALE = 1.0 / math.sqrt(HD)

_BUILDS: dict = {}
LAST_RESULT = None


def _build_causal():
    nc = bacc.Bacc("TRN2", target_bir_lowering=False, debug=False,
                   num_devices=N_CORES)

    xt_d = nc.dram_tensor("xt", [NJ, 128, DCH, SC], BF, kind="ExternalInput").ap()
    wq_d = nc.dram_tensor("wq", [128, DCH, N_LOC], BF, kind="ExternalInput").ap()
    wk_d = nc.dram_tensor("wk", [128, DCH, N_LOC], BF, kind="ExternalInput").ap()
    wv_d = nc.dram_tensor("wv", [128, DCH, N_LOC], BF, kind="ExternalInput").ap()
    wo_d = nc.dram_tensor("wo", [128, H_LOC, D], BF, kind="ExternalInput").ap()
    cose_d = nc.dram_tensor("cose", [128, S], BF, kind="ExternalInput").ap()
    sine_d = nc.dram_tensor("sine", [128, S], BF, kind="ExternalInput").ap()
    tri01_d = nc.dram_tensor("tri01", [128, 128], BF, kind="ExternalInput").ap()
    out_d = nc.dram_tensor("out", [S, D], BF, kind="ExternalOutput").ap()
    out_v = out_d.rearrange("(a p) d -> a p d", p=128)

    with tile.TileContext(nc) as tc:
        with (
            tc.tile_pool(name="singles", bufs=1) as singles,
            tc.tile_pool(name="doubles", bufs=2) as doubles,
            tc.tile_pool(name="triples", bufs=3) as triples,
            tc.tile_pool(name="quads", bufs=4) as quads,
            tc.tile_pool(name="ps1", bufs=1, space="PSUM") as ps1,
            tc.tile_pool(name="ps2", bufs=2, space="PSUM") as ps2,
        ):
            # ---- persistent tensors ----
            wq_sb = singles.tile([128, DCH, N_LOC], BF, tag="wq")
            wk_sb = singles.tile([128, DCH, N_LOC], BF, tag="wk")
            wv_sb = singles.tile([128, DCH, N_LOC], BF, tag="wv")
            wo_sb = singles.tile([128, H_LOC, D], BF, tag="wo")
            cose_sb = singles.tile([128, S], BF, tag="cose")
            sine_sb = singles.tile([128, S], BF, tag="sine")
            tri01_sb = singles.tile([128, 128], BF, tag="tri01")
            ktrot = singles.tile([128, H_LOC, S], BF, tag="ktrot")
            v_sb = singles.tile([128, NJ * H_LOC, SC], BF, tag="v")
            xt0_sb = doubles.tile([128, DCH, SC], BF, tag="xt")

            # DMA emission in consumption order: chunk-0 q/k run d-outer with
            # wq on the sync queue and wk on the (otherwise idle) scalar
            # queue so both stream in parallel; first pieces small so the PE
            # starts early; xt chunk 0 on gpsimd with matching pieces
            PIECES = ((0, 1), (1, 2), (2, 3), (3, 5), (5, 8), (8, 12), (12, 16))
            for a, b in PIECES:
                nc.sync.dma_start(out=wq_sb[:, a:b, :], in_=wq_d[:, a:b, :])
                nc.scalar.dma_start(out=wk_sb[:, a:b, :], in_=wk_d[:, a:b, :])
                nc.gpsimd.dma_start(out=xt0_sb[:, a:b, :], in_=xt_d[0][:, a:b, :])
            nc.sync.dma_start(out=cose_sb[:, 0:SC], in_=cose_d[:, 0:SC])
            nc.sync.dma_start(out=sine_sb[:, 0:SC], in_=sine_d[:, 0:SC])
            nc.sync.dma_start(out=tri01_sb[:], in_=tri01_d[:])
            nc.sync.dma_start(out=wv_sb[:, 0:8, :], in_=wv_d[:, 0:8, :])
            nc.sync.dma_start(out=wv_sb[:, 8:16, :], in_=wv_d[:, 8:16, :])
            nc.sync.dma_start(out=cose_sb[:, SC:], in_=cose_d[:, SC:])
            nc.sync.dma_start(out=sine_sb[:, SC:], in_=sine_d[:, SC:])
            nc.sync.dma_start(out=wo_sb[:], in_=wo_d[:])
            ones_sb = singles.tile([128, 128], BF, tag="ones")
            nc.vector.memset(ones_sb[:], 1.0)

            # ---- PE warmup: the HAM clock gate starts at 1.2GHz and only
            # reaches 2.4GHz after ~3.4us of sustained matmul activity. The
            # PE sits idle during the initial DMA ramp anyway, so burn that
            # time on dummy matmuls and take the cold penalty for free.
            warm_sb = singles.tile([128, SC], BF, tag="warm")
            nc.vector.memset(warm_sb[:], 0.0)
            warm_ps = ps2.tile([128, 2, SC], F32, tag="sc", name="warm_ps")
            for _ in range(12):
                nc.tensor.matmul(warm_ps[:, 0, :], ones_sb[:], warm_sb[:],
                                 start=True, stop=True)

            def rope_group(items):
                """items: list of (psum AP, dest AP, js). Emits all muls (and
                swap DMAs) before any add so the DVE never head-of-line
                blocks a PSUM-freeing mul behind a DMA-waiting add."""
                pend = []
                for ps_ap, dst, js in items:
                    a_sb = quads.tile([128, SC], BF, tag="ropeA")
                    nc.vector.tensor_mul(a_sb[:], ps_ap, cose_sb[:, js:js + SC])
                    b_sb = quads.tile([128, SC], BF, tag="ropeB")
                    nc.vector.tensor_mul(b_sb[:], ps_ap, sine_sb[:, js:js + SC])
                    b2_sb = quads.tile([128, SC], BF, tag="ropeB2")
                    nc.gpsimd.dma_start(out=b2_sb[0:64, :], in_=b_sb[64:128, :])
                    nc.gpsimd.dma_start(out=b2_sb[64:128, :], in_=b_sb[0:64, :])
                    pend.append((dst, a_sb, b2_sb))
                for dst, a_sb, b2_sb in pend:
                    nc.vector.tensor_add(dst, a_sb[:], b2_sb[:])

            def proj_qk_pair(j, xt_sb, w_sb, hp, qdest):
                """q or k projections + rope for one head pair of chunk j."""
                js = j * SC
                items = []
                for hh in range(2):
                    h = hp * 2 + hh
                    ps = ps2.tile([128, SC], F32, tag="qkv1")
                    for d in range(DCH):
                        nc.tensor.matmul(
                            ps[:],
                            w_sb[:, d, h * 128:(h + 1) * 128],
                            xt_sb[:, d, :],
                            start=(d == 0), stop=(d == DCH - 1),
                        )
                    if qdest is not None:
                        dst = qdest[:, h, :]
                    else:
                        dst = ktrot[:, h, js:js + SC]
                    items.append((ps[:], dst, js))
                rope_group(items)

            def proj_v(j, xt_sb):
                for tl in range(4):
                    ps = ps2.tile([128, SC], F32, tag="qkv1")
                    for d in range(DCH):
                        nc.tensor.matmul(
                            ps[:],
                            xt_sb[:, d, tl * 128:(tl + 1) * 128],
                            wv_sb[:, d, :],
                            start=(d == 0), stop=(d == DCH - 1),
                        )
                    nc.scalar.copy(out=v_sb[:, 4 * j + tl, :], in_=ps[:])

            def wo_units(j, attnT_j, st, dcs, deep=False):
                # stage the s-block's 4 units in one tile: a single 512KB DMA
                # with 4KB-contiguous runs per partition instead of 4 strided
                # 128KB DMAs
                o4 = triples.tile([128, 4, SC], BF, tag="ostage")
                for dc in dcs:
                    if deep and dc % 2 == 1:
                        # end-of-kernel block: no attention running, so the
                        # sc banks are free — use them for a deeper pipeline
                        wide = ps2.tile([128, 2, SC], F32, tag="sc",
                                        name="wo_wide")
                        wps = wide[:, 0, :]
                    else:
                        wnarrow = ps2.tile([128, SC], F32, tag="qkv1",
                                           name="wo_wps")
                        wps = wnarrow[:]
                    for h2 in range(H_LOC):
                        nc.tensor.matmul(
                            wps,
                            attnT_j[:, h2, st * 128:(st + 1) * 128],
                            wo_sb[:, h2, dc * SC:(dc + 1) * SC],
                            start=(h2 == 0), stop=(h2 == H_LOC - 1),
                        )
                    if (st + dc) % 2 == 0:
                        nc.scalar.copy(out=o4[:, dc, :], in_=wps)
                    else:
                        nc.vector.tensor_copy(o4[:, dc, :], wps)
                    if deep:
                        # final block: per-unit DMAs so the drain after the
                        # last matmul is one 128KB transfer, not 512KB
                        nc.sync.dma_start(
                            out=out_v[4 * j + st][:, dc * SC:(dc + 1) * SC],
                            in_=o4[:, dc, :])
                if not deep:
                    nc.sync.dma_start(out=out_v[4 * j + st], in_=o4[:])

            def att_head(j, h, qt, attnT_j):
                """Attention for head h of chunk j. qt: [128, SC] rotated q.

                Diagonal tiles first (longest chains), column-trimmed to the
                causal extent; triangular edge masked by a {0,1} multiply on
                DVE after exp. Full tiles follow at full width.
                """
                sums_ps = ps1.tile([128, SC], F32, tag="sums")
                pv_ps = ps1.tile([128, SC], F32, tag="pv")
                n_fp = 2 * j  # full (off-diagonal) t-tile pairs
                dexp = []
                for half in range(2):
                    sc_ps = ps2.tile([128, 2, SC], F32, tag="sc")
                    exp_sb = triples.tile([128, 2, SC], BF, tag="exp")
                    for i_ in range(2):
                        p = 2 * half + i_
                        tt = 4 * j + p
                        w0 = p * 128
                        nc.tensor.matmul(
                            sc_ps[:, i_, w0:SC],
                            ktrot[:, h, tt * 128:(tt + 1) * 128],
                            qt[:, w0:SC],
                            start=True, stop=True,
                        )
                        if p < 3:
                            nc.scalar.activation(
                                out=exp_sb[:, i_, w0 + 128:SC],
                                in_=sc_ps[:, i_, w0 + 128:SC],
                                func=AF.Exp, scale=SCALE)
                        bt = quads.tile([128, 128], BF, tag="bt")
                        nc.scalar.activation(
                            out=bt[:], in_=sc_ps[:, i_, w0:w0 + 128],
                            func=AF.Exp, scale=SCALE)
                        nc.vector.tensor_mul(
                            exp_sb[:, i_, w0:w0 + 128], bt[:], tri01_sb[:])
                    for i_ in range(2):
                        p = 2 * half + i_
                        tt = 4 * j + p
                        w0 = p * 128
                        nc.tensor.matmul(
                            pv_ps[:, w0:SC],
                            v_sb[:, tt, h * 128:(h + 1) * 128],
                            exp_sb[:, i_, w0:SC],
                            start=(p == 0), stop=(j == 0 and p == 3),
                            skip_group_check=True,
                        )
                    dexp.append(exp_sb)
                # piecewise exp-sum of the trimmed diag tiles (valid columns
                # of tile p are [128p:512]; untouched regions hold garbage)
                t01 = triples.tile([128, SC], BF, tag="t01")
                nc.vector.tensor_copy(t01[:, 0:128], dexp[0][:, 0, 0:128])
                nc.vector.tensor_add(t01[:, 128:SC], dexp[0][:, 0, 128:SC],
                                     dexp[0][:, 1, 128:SC])
                t23 = triples.tile([128, SC], BF, tag="t23")
                nc.vector.tensor_copy(t23[:, 256:384], dexp[1][:, 0, 256:384])
                nc.vector.tensor_add(t23[:, 384:SC], dexp[1][:, 0, 384:SC],
                                     dexp[1][:, 1, 384:SC])
                # single full-width ones-matmul: a second start=True subrange
                # piece on the same bank would reset has_written bank-wide
                # and break later accumulation
                efin = triples.tile([128, SC], BF, tag="efin")
                nc.vector.tensor_copy(efin[:, 0:256], t01[:, 0:256])
                nc.vector.tensor_add(efin[:, 256:SC], t01[:, 256:SC],
                                     t23[:, 256:SC])
                nc.tensor.matmul(sums_ps[:], ones_sb[:], efin[:],
                                 start=True, stop=(j == 0),
                                 skip_group_check=True)
                # ---- full tiles ----
                prev_epair = None
                for fp in range(n_fp):
                    sc_ps = ps2.tile([128, 2, SC], F32, tag="sc")
                    exp_sb = triples.tile([128, 2, SC], BF, tag="exp")
                    for i_ in range(2):
                        tt = fp * 2 + i_
                        nc.tensor.matmul(
                            sc_ps[:, i_, :],
                            ktrot[:, h, tt * 128:(tt + 1) * 128],
                            qt[:],
                            start=True, stop=True,
                        )
                    nc.scalar.activation(out=exp_sb[:], in_=sc_ps[:],
                                         func=AF.Exp, scale=SCALE)
                    epair = triples.tile([128, SC], BF, tag="epair")
                    nc.vector.tensor_add(epair[:], exp_sb[:, 0, :],
                                         exp_sb[:, 1, :])
                    if fp % 2 == 0:
                        prev_epair = epair
                    else:
                        equad = triples.tile([128, SC], BF, tag="equad")
                        nc.vector.tensor_add(equad[:], prev_epair[:], epair[:])
                        nc.tensor.matmul(sums_ps[:], ones_sb[:], equad[:],
                                         start=False, stop=(fp == n_fp - 1),
                                         skip_group_check=True)
                    for i_ in range(2):
                        tt = fp * 2 + i_
                        nc.tensor.matmul(
                            pv_ps[:],
                            v_sb[:, tt, h * 128:(h + 1) * 128],
                            exp_sb[:, i_, :],
                            start=False, stop=(fp == n_fp - 1 and i_ == 1),
                            skip_group_check=True,
                        )
                recip_sb = doubles.tile([128, SC], F32, tag="recip")
                nc.vector.reciprocal_approx_fast(out=recip_sb[:], in_=sums_ps[:])
                nc.vector.tensor_mul(attnT_j[:, h, :], pv_ps[:], recip_sb[:])

            # ---- chunk 0 projections in two waves so the PE never waits on
            # the serial rope chains: wave A (q x4 + k h0/h1) runs d-outer
            # over 6 PSUM banks tracking the streaming DMA pieces; wave B
            # (k h2/h3) lands in the still-free second sc buffer and keeps
            # the PE busy while wave A's ropes drain on DVE ----
            qps = [ps2.tile([128, SC], F32, tag="qkv1", name="qps0"),
                   ps2.tile([128, SC], F32, tag="qkv1", name="qps1"),
                   ps1.tile([128, SC], F32, tag="sums", name="qps2"),
                   ps1.tile([128, SC], F32, tag="pv", name="qps3")]
            kpsA = ps2.tile([128, 2, SC], F32, tag="sc", name="kpsA")
            for d in range(DCH):
                for h in range(H_LOC):
                    nc.tensor.matmul(qps[h][:],
                                     wq_sb[:, d, h * 128:(h + 1) * 128],
                                     xt0_sb[:, d, :],
                                     start=(d == 0), stop=(d == DCH - 1))
                for h in range(2):
                    nc.tensor.matmul(kpsA[:, h, :],
                                     wk_sb[:, d, h * 128:(h + 1) * 128],
                                     xt0_sb[:, d, :],
                                     start=(d == 0), stop=(d == DCH - 1))
                if 1 <= d <= 8:
                    # filler during DMA piece waits: keeps the HAM activity
                    # window busy so the PE clock never re-throttles while
                    # wave A is input-bandwidth paced
                    nc.tensor.matmul(warm_ps[:, 0, :], ones_sb[:], warm_sb[:],
                                     start=True, stop=True)
            qtrot0 = doubles.tile([128, H_LOC, SC], BF, tag="qtrot")
            rope_group([(qps[h][:], qtrot0[:, h, :], 0) for h in range(H_LOC)])
            rope_group([(kpsA[:, h, :], ktrot[:, h, 0:SC], 0)
                        for h in range(2)])
            kpsB = ps2.tile([128, 2, SC], F32, tag="sc", name="kpsB")
            for d in range(DCH):
                for h in range(2):
                    nc.tensor.matmul(kpsB[:, h, :],
                                     wk_sb[:, d, (h + 2) * 128:(h + 3) * 128],
                                     xt0_sb[:, d, :],
                                     start=(d == 0), stop=(d == DCH - 1))
            rope_group([(kpsB[:, h, :], ktrot[:, h + 2, 0:SC], 0)
                        for h in range(2)])
            proj_v(0, xt0_sb)

            # ---- attention chunks with interleaved next-chunk projections
            # and previous-chunk wo units filling softmax-chain bubbles ----
            prev_attnT = None
            xt_cur, qt_cur = xt0_sb, qtrot0
            for j in range(NJ):
                attnT_j = doubles.tile([128, H_LOC, SC], BF, tag="attnT")
                if j + 1 < NJ:
                    xt_next = doubles.tile([128, DCH, SC], BF, tag="xt")
                    for a, b in ((0, 4), (4, 8), (8, 12), (12, 16)):
                        nc.gpsimd.dma_start(out=xt_next[:, a:b, :],
                                            in_=xt_d[j + 1][:, a:b, :])
                    qt_next = doubles.tile([128, H_LOC, SC], BF, tag="qtrot")
                for h in range(H_LOC):
                    att_head(j, h, qt_cur[:, h, :], attnT_j)
                    if prev_attnT is not None:
                        wo_units(j - 1, prev_attnT, h, range(4))
                    if j + 1 < NJ:
                        if h < 2:
                            proj_qk_pair(j + 1, xt_next, wq_sb, h, qt_next)
                        else:
                            proj_qk_pair(j + 1, xt_next, wk_sb, h - 2, None)
                if j + 1 < NJ:
                    proj_v(j + 1, xt_next)
                    xt_cur, qt_cur = xt_next, qt_next
                prev_attnT = attnT_j
            for st in range(4):
                wo_units(NJ - 1, prev_attnT, st, range(4), deep=True)

    nc.compile()
    return nc


def _build_legacy(variant: str, nj: int = NJ):
    """variant in {'full_nomask', 'full_mask'} (original implementation)."""
    use_mask = variant == "full_mask"

    nc = bacc.Bacc("TRN2", target_bir_lowering=False, debug=False,
                   num_devices=N_CORES)

    xt_d = nc.dram_tensor("xt", [NJ, 128, DCH, SC], BF, kind="ExternalInput").ap()
    wq_d = nc.dram_tensor("wq", [128, DCH, N_LOC], BF, kind="ExternalInput").ap()
    wk_d = nc.dram_tensor("wk", [128, DCH, N_LOC], BF, kind="ExternalInput").ap()
    wv_d = nc.dram_tensor("wv", [128, DCH, N_LOC], BF, kind="ExternalInput").ap()
    wo_d = nc.dram_tensor("wo", [128, H_LOC, D], BF, kind="ExternalInput").ap()
    cose_d = nc.dram_tensor("cose", [128, S], F32, kind="ExternalInput").ap()
    sine_d = nc.dram_tensor("sine", [128, S], F32, kind="ExternalInput").ap()
    maskt_d = None
    if use_mask:
        maskt_d = nc.dram_tensor("maskt", [NJ, 128, DCH, SC], BF,
                                 kind="ExternalInput").ap()
    out_d = nc.dram_tensor("out", [S, D], F32, kind="ExternalOutput").ap()
    out_v = out_d.rearrange("(a p) d -> a p d", p=128)

    with tile.TileContext(nc) as tc:
        with (
            tc.tile_pool(name="singles", bufs=1) as singles,
            tc.tile_pool(name="doubles", bufs=2) as doubles,
            tc.tile_pool(name="triples", bufs=3) as triples,
            tc.tile_pool(name="ps1", bufs=1, space="PSUM") as ps1,
            tc.tile_pool(name="ps2", bufs=2, space="PSUM") as ps2,
        ):
            rope_pool = doubles if use_mask else triples
            stage_pool = doubles if use_mask else triples
            epair_pool = doubles
            # ---- constants / persistent tensors ----
            wq_sb = singles.tile([128, DCH, N_LOC], BF, tag="wq")
            wk_sb = singles.tile([128, DCH, N_LOC], BF, tag="wk")
            wv_sb = singles.tile([128, DCH, N_LOC], BF, tag="wv")
            wo_sb = singles.tile([128, H_LOC, D], BF, tag="wo")
            xt_pool = singles if use_mask else doubles
            xt0_sb = xt_pool.tile([128, DCH, SC], BF, tag="xt")
            for q4 in range(4):
                nc.sync.dma_start(out=wq_sb[:, 4 * q4:4 * (q4 + 1), :],
                                  in_=wq_d[:, 4 * q4:4 * (q4 + 1), :])
                nc.gpsimd.dma_start(out=xt0_sb[:, 4 * q4:4 * (q4 + 1), :],
                                    in_=xt_d[0][:, 4 * q4:4 * (q4 + 1), :])
            cose_sb = singles.tile([128, S], F32, tag="cose")
            sine_sb = singles.tile([128, S], F32, tag="sine")
            nc.sync.dma_start(out=cose_sb[:], in_=cose_d[:])
            nc.sync.dma_start(out=sine_sb[:], in_=sine_d[:])
            nc.sync.dma_start(out=wk_sb[:], in_=wk_d[:])
            nc.sync.dma_start(out=wv_sb[:], in_=wv_d[:])
            nc.sync.dma_start(out=wo_sb[:], in_=wo_d[:])
            from concourse.masks import make_identity
            ones_sb = singles.tile([128, 128], BF, tag="ones")
            nc.vector.memset(ones_sb[:], 1.0)
            ident_sb = singles.tile([128, 128], BF, tag="ident")
            make_identity(nc, ident_sb[:])
            ktrot = singles.tile([128, H_LOC, S], BF, tag="ktrot")
            v_sb = singles.tile([128, NJ * H_LOC, SC], BF, tag="v")
            qtrot_all = singles.tile([128, H_LOC, S], BF, tag="qtrot_all")

            def projections(j, qdest, qsl, xt_pre=None):
                js = j * SC
                if xt_pre is not None:
                    xt_sb = xt_pre
                else:
                    xt_sb = xt_pool.tile([128, DCH, SC], BF, tag="xt")
                    nc.gpsimd.dma_start(out=xt_sb[:], in_=xt_d[j])

                for w_sb, dest, dsl in ((wq_sb, qdest, qsl),
                                        (wk_sb, ktrot, slice(js, js + SC))):
                    for hp in range(2):
                        parts = []
                        for hh in range(2):
                            h = hp * 2 + hh
                            if j == 0 and hp == 1:
                                ps = ps1.tile([128, SC], F32,
                                              tag="pv" if hh else "sums")
                            else:
                                ps = ps2.tile([128, SC], F32, tag="qkv1")
                            for d in range(DCH):
                                nc.tensor.matmul(
                                    ps[:],
                                    w_sb[:, d, h * 128:(h + 1) * 128],
                                    xt_sb[:, d, :],
                                    start=(d == 0), stop=(d == DCH - 1),
                                )
                            a_sb = rope_pool.tile([128, SC], F32, tag="ropeA")
                            nc.vector.tensor_mul(
                                a_sb[:], ps[:], cose_sb[:, js:js + SC])
                            b_sb = triples.tile([128, SC], F32, tag="ropeB")
                            nc.vector.tensor_mul(
                                b_sb[:], ps[:], sine_sb[:, js:js + SC])
                            b2_sb = triples.tile([128, SC], F32, tag="ropeB2")
                            nc.scalar.dma_start(out=b2_sb[0:64, :],
                                                in_=b_sb[64:128, :])
                            nc.scalar.dma_start(out=b2_sb[64:128, :],
                                                in_=b_sb[0:64, :])
                            parts.append((h, a_sb, b2_sb))
                        for h, a_sb, b2_sb in parts:
                            if dsl is None:
                                dst = dest[:, h, :]
                            else:
                                dst = dest[:, h, dsl]
                            nc.vector.tensor_add(dst, a_sb[:], b2_sb[:])

                for tl in range(4):
                    ps = ps2.tile([128, SC], F32, tag="qkv1")
                    for d in range(DCH):
                        nc.tensor.matmul(
                            ps[:],
                            xt_sb[:, d, tl * 128:(tl + 1) * 128],
                            wv_sb[:, d, :],
                            start=(d == 0), stop=(d == DCH - 1),
                        )
                    nc.scalar.copy(out=v_sb[:, 4 * j + tl, :], in_=ps[:])

            def wo_units(j, attnT_j, st, dcs):
                for dc in dcs:
                    wps = ps2.tile([128, SC], F32, tag="qkv1")
                    for h2 in range(H_LOC):
                        nc.tensor.matmul(
                            wps[:],
                            attnT_j[:, h2, st * 128:(st + 1) * 128],
                            wo_sb[:, h2, dc * SC:(dc + 1) * SC],
                            start=(h2 == 0), stop=(h2 == H_LOC - 1),
                        )
                    o_sb = stage_pool.tile([128, SC], F32, tag="ostage")
                    if (st + dc) % 2 == 0:
                        nc.scalar.copy(out=o_sb[:], in_=wps[:])
                    else:
                        nc.vector.tensor_copy(o_sb[:], wps[:])
                    nc.sync.dma_start(
                        out=out_v[4 * j + st][:, dc * SC:(dc + 1) * SC],
                        in_=o_sb[:])

            def attention_and_wo(j, qtrot_h, prev=None):
                maskt_sb = None
                if use_mask:
                    maskt_sb = xt_pool.tile([128, DCH, SC], BF, tag="xt")
                    nc.sync.dma_start(out=maskt_sb[:], in_=maskt_d[j])

                attnT_j = doubles.tile([128, H_LOC, SC], BF, tag="attnT")
                pg_order = list(range(DCH // 2))
                for h in range(H_LOC):
                    sums_ps = ps1.tile([128, SC], F32, tag="sums")
                    pv_ps = ps1.tile([128, SC], F32, tag="pv")
                    for gi, pg in enumerate(pg_order):
                        sc_ps = ps2.tile([128, 2, SC], F32, tag="sc")
                        exp_sb = stage_pool.tile([128, 2, SC], BF, tag="exp")
                        for i_ in range(2):
                            tt = pg * 2 + i_
                            nc.tensor.matmul(
                                sc_ps[:, i_, :],
                                ktrot[:, h, tt * 128:(tt + 1) * 128],
                                qtrot_h(h),
                                start=True, stop=not use_mask,
                            )
                            if use_mask:
                                nc.tensor.matmul(
                                    sc_ps[:, i_, :], ident_sb[:],
                                    maskt_sb[:, tt, :],
                                    start=False, stop=True,
                                )
                        nc.scalar.activation(out=exp_sb[:], in_=sc_ps[:],
                                             func=AF.Exp, scale=SCALE)
                        epair = epair_pool.tile([128, SC], BF, tag="epair")
                        nc.vector.tensor_add(epair[:], exp_sb[:, 0, :],
                                             exp_sb[:, 1, :])
                        nc.tensor.matmul(sums_ps[:], ones_sb[:], epair[:],
                                         start=gi == 0,
                                         stop=gi == len(pg_order) - 1)
                        for i_ in range(2):
                            tt = pg * 2 + i_
                            first = gi == 0 and i_ == 0
                            last = gi == len(pg_order) - 1 and i_ == 1
                            nc.tensor.matmul(pv_ps[:],
                                             v_sb[:, tt, h * 128:(h + 1) * 128],
                                             exp_sb[:, i_, :],
                                             start=first, stop=last)
                    recip_sb = doubles.tile([128, SC], F32, tag="recip")
                    nc.vector.reciprocal_approx_fast(out=recip_sb[:], in_=sums_ps[:])
                    nc.vector.tensor_mul(attnT_j[:, h, :], pv_ps[:], recip_sb[:])
                    if prev is not None:
                        wo_units(j - 1, prev, h, range(4))
                return attnT_j

            pending = None
            for j in range(nj):
                projections(j, qtrot_all, slice(j * SC, (j + 1) * SC),
                            xt_pre=xt0_sb if j == 0 else None)
            for j in range(nj):
                js = j * SC
                pending = attention_and_wo(
                    j, lambda h, js=js: qtrot_all[:, h, js:js + SC],
                    prev=pending)
            if pending is not None:
                for st in range(4):
                    wo_units(nj - 1, pending, st, range(4))

    nc.compile()
    return nc


def _get_build(variant):
    if variant not in _BUILDS:
        if variant == "causal":
            _BUILDS[variant] = _build_causal()
        else:
            _BUILDS[variant] = _build_legacy(variant)
    return _BUILDS[variant]


def _classify_mask(mask):
    if not np.any(mask):
        return "full_nomask"
    tril = np.tril(np.ones((S, S), dtype=bool))
    if np.all(mask[tril] == 0.0) and np.all(mask[~tril] <= -1e9):
        return "causal"
    return "full_mask"


def kernel(x, wq, wk, wv, wo, freqs_cos, freqs_sin, mask):
    global LAST_RESULT
    x = np.asarray(x)
    wq, wk, wv, wo = (np.asarray(w) for w in (wq, wk, wv, wo))
    freqs_cos = np.asarray(freqs_cos, dtype=np.float32)
    freqs_sin = np.asarray(freqs_sin, dtype=np.float32)
    mask = np.asarray(mask, dtype=np.float32)

    variant = _classify_mask(mask)
    causal = variant == "causal"
    nc = _get_build(variant)

    # half-layout column permutation within each head (even indices then odd)
    perm = np.concatenate([np.arange(0, 128, 2), np.arange(1, 128, 2)])

    def wproj_arr(w, g):
        cols = w[:, 512 * g:512 * (g + 1)].reshape(D, H_LOC, 128)
        cols = cols[:, :, perm].reshape(D, N_LOC)
        return np.ascontiguousarray(
            cols.reshape(DCH, 128, N_LOC).transpose(1, 0, 2)).astype(BF16)

    def wv_arr(w, g):
        cols = w[:, 512 * g:512 * (g + 1)]
        return np.ascontiguousarray(
            cols.reshape(DCH, 128, N_LOC).transpose(1, 0, 2)).astype(BF16)

    def wo_arr(g):
        rows = wo[512 * g:512 * (g + 1), :]
        return np.ascontiguousarray(
            rows.reshape(H_LOC, 128, D).transpose(1, 0, 2)).astype(BF16)

    # cos/sin in half-layout: rows j and j+64 carry pair j's cos; sine rows
    # 0..63 = +sin (source a_j -> target j+64), rows 64..127 = -sin
    cs_dt = BF16 if causal else np.float32
    cosE = np.empty((128, S), cs_dt)
    sinE = np.empty((128, S), cs_dt)
    cosE[0:64] = freqs_cos.T
    cosE[64:128] = freqs_cos.T
    sinE[0:64] = freqs_sin.T
    sinE[64:128] = -freqs_sin.T

    xt_b = []
    for b in range(B):
        xT = x[b].T.astype(BF16)  # [D, S]
        xt = np.ascontiguousarray(
            xT.reshape(DCH, 128, NJ, SC).transpose(2, 1, 0, 3))
        xt_b.append(xt)

    tri01 = None
    if causal:
        r = np.arange(128)
        tri01 = (r[:, None] <= r[None, :]).astype(BF16)

    maskt = None
    if variant == "full_mask":
        # exp computes exp(SCALE * (scores + m')) with m' = mask^T / SCALE
        mT = (mask.T / SCALE).astype(BF16)  # [t, s]
        maskt = np.ascontiguousarray(
            mT.reshape(DCH, 128, NJ, SC).transpose(2, 1, 0, 3))

    wq_g = [wproj_arr(wq, g) for g in range(H_LOC)]
    wk_g = [wproj_arr(wk, g) for g in range(H_LOC)]
    wv_g = [wv_arr(wv, g) for g in range(H_LOC)]
    wo_g = [wo_arr(g) for g in range(H_LOC)]

    in_maps = []
    for c in range(N_CORES):
        b, g = c // 4, c % 4
        m = {
            "xt": xt_b[b],
            "wq": wq_g[g], "wk": wk_g[g], "wv": wv_g[g], "wo": wo_g[g],
            "cose": cosE, "sine": sinE,
        }
        if tri01 is not None:
            m["tri01"] = tri01
        if maskt is not None:
            m["maskt"] = maskt
        in_maps.append(m)

    res = run_bass_kernel_spmd(nc, in_maps, list(range(N_CORES)))
    LAST_RESULT = res

    outs = [np.asarray(res.results[c]["out"], dtype=np.float32)
            for c in range(N_CORES)]
    out = np.stack([
        outs[0] + outs[1] + outs[2] + outs[3],
        outs[4] + outs[5] + outs[6] + outs[7],
    ])
    return out
